# revision 16
# baseline (speedup 1.0000x reference)
"""Trainium2 Bass kernel for nn_ConditionalMoELayer (expert-parallel, sparse).

Design: the routing pipeline (difficulty net -> k, gate -> top-k softmax) is
tiny (~3% of FLOPs) and entirely sequential with the dispatch decision, so it
runs on the host in fp64 — bit-identical routing decisions to the fp32
reference (decision boundaries sit ~1e-1 apart while fp32-vs-fp64 z noise is
~1e-6). The host then groups tokens by assigned expert ("all-to-all"), and
ONE SPMD device launch runs the expert FFNs, which is >97% of the FLOPs:

  cores 2e, 2e+1 hold expert e's weights; each processes half of expert e's
  assigned tokens (C = max over experts of ceil(n_e/2), zero-padded).

The FFN runs in bf16 (matmul full rate, 1 cycle/row — same as fp32r at
>=256-wide moving operands, but half the HBM traffic and no small-tile rate
cliff on the tail chunk). W1 and W2 both stay resident in SBUF (64KB of the
208KB partition budget), so HBM reads are just 11.5MB/core vs compute of
~183us — DMA fully hidden. PSUM accumulates in fp32; only operand storage is
bf16, so the output error is ~3e-3 relative, far inside the 2e-2 gate.

Host combine: out[tok] += dense_w * y (token sets within one core are
disjoint), plus dense_w @ eb2.

Only ~40% of token-expert pairs are active, so this does ~2.5x fewer FLOPs
than the dense equivalent.
"""

import numpy as np
import ml_dtypes

BF16 = np.dtype(ml_dtypes.bfloat16)

P = 128          # partitions
D = 1024         # d_model
H = 2048         # expert hidden
E = 4            # experts
T_FULL = 8192    # total tokens
N_CORES = 8

KD = D // P      # 8   k-subtiles over D
KH = H // P      # 16  k-subtiles over H
ND = D // P      # 8   d-tiles (FFN output)
TCH = 512        # token chunk for matmul moving operand (1 PSUM bank of f32)

TH_LO = 0.5
TH_HI = 2.0
MIN_E = 1


def _ap_name(t):
    return t.tensor.name if hasattr(t, "tensor") else t.name


def build_ffn_nc(C):
    """One expert FFN over C gathered tokens per core, bf16 operands.

    mm1 computes hid^T = relu(W1^T x^T + b1), mm2 computes y^T = W2^T hid^T
    (weights stationary, tokens moving). W1, W2 fully resident in SBUF;
    x streams in per chunk on the gpsimd queue, y streams out behind it.
    """
    import concourse.mybir as mybir
    import concourse.tile as tile
    from concourse import bacc
    from contextlib import ExitStack

    f32 = mybir.dt.float32
    bf16 = mybir.dt.bfloat16
    AF = mybir.ActivationFunctionType

    # chunk widths: a 448-wide first chunk (small enough that its x lands
    # early and the first matmul fires ~12us in, large enough that mm1/mm2
    # consumption stays behind the weight stream), then 512s, with the
    # remainder arranged so no chunk is narrower than ~180 tokens (narrow
    # chunks approach the PE's LDWEIGHTS issue floor)
    widths = [min(448, C)]
    rem = C - widths[0]
    while rem > TCH + 180:
        widths.append(TCH)
        rem -= TCH
    if rem > TCH:
        widths.extend([(rem + 1) // 2, rem // 2])
    elif rem:
        widths.append(rem)
    chunks = []
    c0 = 0
    for cw in widths:
        chunks.append((c0, cw))
        c0 += cw

    nc = bacc.Bacc(None, target_bir_lowering=False, debug=False)

    with tile.TileContext(nc) as tc:
        with ExitStack() as ctx:
            dram = ctx.enter_context(tc.tile_pool(name="dram", bufs=1, space="DRAM"))
            xg_d = dram.tile([P, KD, C], bf16, kind="ExternalInput", name="xg")
            w1_d = dram.tile([P, KH, KD, P], bf16, kind="ExternalInput", name="w1g")
            w2_d = dram.tile([P, ND, KH, P], bf16, kind="ExternalInput", name="w2g")
            eb1_d = dram.tile([P, KH], f32, kind="ExternalInput", name="eb1g")
            yt_d = dram.tile([P, ND, C], bf16, kind="ExternalOutput", name="yt")

            const = ctx.enter_context(tc.tile_pool(name="const", bufs=1))
            xgp = ctx.enter_context(tc.tile_pool(name="xgp", bufs=1))
            hidp = ctx.enter_context(tc.tile_pool(name="hidp", bufs=2))
            ytp = ctx.enter_context(tc.tile_pool(name="ytp", bufs=2))
            psb = ctx.enter_context(tc.tile_pool(name="psb", bufs=3, space="PSUM"))
            ps2p = ctx.enter_context(tc.tile_pool(name="ps2p", bufs=3, space="PSUM"))

            w1_sb = const.tile([P, KH, KD, P], bf16, tag="w1")
            w2_sb = const.tile([P, ND, KH, P], bf16, tag="w2")
            eb1_sb = const.tile([P, KH], f32, tag="eb1")
            xg_sb = xgp.tile([P, KD, C], bf16, tag="xg")

            # DMA-completion semaphores post when the DATA actually lands,
            # and the two DMA queues carry independent bandwidth — so split
            # inputs across both, each in its consumers' order: activations
            # (and the tiny bias) on gpsimd, weights on sync. The first
            # matmul fires once xg chunk 0 + W1[ht0] land (~14us); after
            # that both streams stay ahead of the PE.
            (c00, cw0) = chunks[0]
            nc.sync.dma_start(xg_sb[:, :, :cw0], xg_d[:, :, :cw0])
            nc.gpsimd.dma_start(eb1_sb[:], eb1_d[:])
            for ht in range(KH):
                nc.sync.dma_start(w1_sb[:, ht], w1_d[:, ht])
            for dt in range(ND):
                nc.sync.dma_start(w2_sb[:, dt], w2_d[:, dt])
            for (c0, cw) in chunks[1:]:
                nc.gpsimd.dma_start(
                    xg_sb[:, :, c0:c0 + cw], xg_d[:, :, c0:c0 + cw]
                )

            for ci, (c0, cw) in enumerate(chunks):
                csl = slice(c0, c0 + cw)
                hidT = hidp.tile([P, KH, TCH], bf16, tag="hid")
                for ht in range(KH):
                    ps = psb.tile([P, TCH], f32, tag="psb")
                    for ko in range(KD):
                        nc.tensor.matmul(
                            ps[:, :cw],
                            w1_sb[:, ht, ko, :],
                            xg_sb[:, ko, csl],
                            start=(ko == 0),
                            stop=(ko == KD - 1),
                        )
                    nc.scalar.activation(
                        hidT[:, ht, :cw], ps[:, :cw], AF.Relu,
                        bias=eb1_sb[:, ht:ht + 1],
                    )
                last = ci == len(chunks) - 1
                yt_sb = ytp.tile([P, ND, TCH], bf16, tag="yt")
                for dt in range(ND):
                    ps2 = ps2p.tile([P, TCH], f32, tag="ps2")
                    for ko in range(KH):
                        nc.tensor.matmul(
                            ps2[:, :cw],
                            w2_sb[:, dt, ko, :],
                            hidT[:, ko, :cw],
                            start=(ko == 0),
                            stop=(ko == KH - 1),
                        )
                    nc.vector.tensor_copy(yt_sb[:, dt, :cw], ps2[:, :cw])
                    if last:
                        # tail: stream each dt-tile out as soon as it's cast
                        # so the final descriptor (and its completion post)
                        # is tiny; sync queue — its end-of-program drain is
                        # cheap where gpsimd's costs ~2.5us
                        nc.sync.dma_start(yt_d[:, dt, csl],
                                          yt_sb[:, dt, :cw])
                if not last:
                    nc.sync.dma_start(yt_d[:, :, csl], yt_sb[:, :, :cw])

    nc.compile()
    names = {k: _ap_name(v) for k, v in {
        "xg": xg_d, "w1g": w1_d, "w2g": w2_d, "eb1g": eb1_d, "yt": yt_d,
    }.items()}
    return nc, names


def build_ffn_seg_nc(qs):
    """Balanced variant: every core runs ALL experts, expert e over q_e =
    ceil(n_e/8) tokens (one <=512-wide chunk per expert segment). Expert
    weights stream per segment through double-buffered SBUF tiles; the
    sync-queue descriptor for segment e+2's W1 parks until segment e's mm1
    frees its buffer, which is harmless because it still lands a full
    segment early. Per-core tokens drop from max_e ceil(n_e/2) to
    sum_e ceil(n_e/8) — perfect load balance across the 8 cores.
    """
    import concourse.mybir as mybir
    import concourse.tile as tile
    from concourse import bacc
    from contextlib import ExitStack

    f32 = mybir.dt.float32
    bf16 = mybir.dt.bfloat16
    AF = mybir.ActivationFunctionType

    CS = sum(qs)
    offs = [sum(qs[:e]) for e in range(E)]

    nc = bacc.Bacc(None, target_bir_lowering=False, debug=False)

    with tile.TileContext(nc) as tc:
        with ExitStack() as ctx:
            dram = ctx.enter_context(tc.tile_pool(name="dram", bufs=1, space="DRAM"))
            xg_d = dram.tile([P, KD, CS], bf16, kind="ExternalInput", name="xg")
            w1_d = dram.tile([E, P, KH, KD, P], bf16, kind="ExternalInput", name="w1g")
            w2_d = dram.tile([E, P, ND, KH, P], bf16, kind="ExternalInput", name="w2g")
            eb1_d = dram.tile([P, E * KH], f32, kind="ExternalInput", name="eb1g")
            yt_d = dram.tile([P, ND, CS], bf16, kind="ExternalOutput", name="yt")

            const = ctx.enter_context(tc.tile_pool(name="const", bufs=1))
            w1p = ctx.enter_context(tc.tile_pool(name="w1p", bufs=2))
            w2p = ctx.enter_context(tc.tile_pool(name="w2p", bufs=2))
            xgp = ctx.enter_context(tc.tile_pool(name="xgp", bufs=1))
            hidp = ctx.enter_context(tc.tile_pool(name="hidp", bufs=1))
            ytp = ctx.enter_context(tc.tile_pool(name="ytp", bufs=2))
            psb = ctx.enter_context(tc.tile_pool(name="psb", bufs=3, space="PSUM"))
            ps2p = ctx.enter_context(tc.tile_pool(name="ps2p", bufs=3, space="PSUM"))
            wps = ctx.enter_context(tc.tile_pool(name="wps", bufs=1, space="PSUM"))

            eb1_sb = const.tile([P, E * KH], f32, tag="eb1")
            xg_sb = xgp.tile([P, KD, CS], bf16, tag="xg")
            w1ts = [w1p.tile([P, KH, KD, P], bf16, tag="w1s", name=f"w1s{e}")
                    for e in range(E)]
            w2ts = [w2p.tile([P, ND, KH, P], bf16, tag="w2s", name=f"w2s{e}")
                    for e in range(E)]

            # sync queue: segment-0 x first (gates the first matmul), then
            # weights in consumption order; y-outs are appended behind by
            # the compute loop. gpsimd queue: bias + remaining x segments.
            nc.sync.dma_start(xg_sb[:, :, :qs[0]], xg_d[:, :, :qs[0]])
            nc.gpsimd.dma_start(eb1_sb[:], eb1_d[:])
            for e in range(E):
                for ht in range(KH):
                    nc.sync.dma_start(w1ts[e][:, ht], w1_d[e, :, ht])
                for dt in range(ND):
                    nc.sync.dma_start(w2ts[e][:, dt], w2_d[e, :, dt])
            for e in range(1, E):
                sl = slice(offs[e], offs[e] + qs[e])
                nc.gpsimd.dma_start(xg_sb[:, :, sl], xg_d[:, :, sl])

            # Warm the PE's p-state during the ~7us wait for the first x/W1
            # data: the clock ramps to 2.4GHz only after ~3us of continuous
            # busy, so burn that ramp on throwaway matmuls over a memset
            # tile. Real matmuls then start at full rate.
            warm_sb = const.tile([P, 16], bf16, tag="warm")
            nc.gpsimd.memset(warm_sb[:], 0.0)
            psw = wps.tile([16, 16], f32, tag="psw")
            for _ in range(56):
                nc.tensor.matmul(psw, warm_sb[:], warm_sb[:],
                                 start=True, stop=True)

            for e in range(E):
                c0, cw = offs[e], qs[e]
                csl = slice(c0, c0 + cw)
                last = e == E - 1
                hidT = hidp.tile([P, KH, TCH], bf16, tag="hid")
                for ht in range(KH):
                    ps = psb.tile([P, TCH], f32, tag="psb")
                    for ko in range(KD):
                        nc.tensor.matmul(
                            ps[:, :cw],
                            w1ts[e][:, ht, ko, :],
                            xg_sb[:, ko, csl],
                            start=(ko == 0),
                            stop=(ko == KD - 1),
                        )
                    nc.scalar.activation(
                        hidT[:, ht, :cw], ps[:, :cw], AF.Relu,
                        bias=eb1_sb[:, e * KH + ht:e * KH + ht + 1],
                    )
                yt_sb = ytp.tile([P, ND, TCH], bf16, tag="yt")
                for dt in range(ND):
                    ps2 = ps2p.tile([P, TCH], f32, tag="ps2")
                    for ko in range(KH):
                        nc.tensor.matmul(
                            ps2[:, :cw],
                            w2ts[e][:, dt, ko, :],
                            hidT[:, ko, :cw],
                            start=(ko == 0),
                            stop=(ko == KH - 1),
                        )
                    nc.vector.tensor_copy(yt_sb[:, dt, :cw], ps2[:, :cw])
                    if last:
                        nc.sync.dma_start(yt_d[:, dt, csl],
                                          yt_sb[:, dt, :cw])
                if not last:
                    nc.sync.dma_start(yt_d[:, :, csl], yt_sb[:, :, :cw])

    nc.compile()
    names = {k: _ap_name(v) for k, v in {
        "xg": xg_d, "w1g": w1_d, "w2g": w2_d, "eb1g": eb1_d, "yt": yt_d,
    }.items()}
    return nc, names


def dispatch_seg(dense_w):
    """Split every expert's token list evenly across all 8 cores."""
    qs = []
    slices = []
    for e in range(E):
        idx = np.nonzero(dense_w[:, e] > 0)[0]
        q = max((len(idx) + N_CORES - 1) // N_CORES, 16)
        qs.append(q)
        slices.append([idx[c * q:(c + 1) * q] for c in range(N_CORES)])
    core_toks = [[slices[e][c] for e in range(E)] for c in range(N_CORES)]
    return core_toks, tuple(qs)


def prep_ffn_seg_in_maps(xf, core_toks, qs, ew1, eb1, ew2):
    xb = xf.astype(BF16)
    CS = sum(qs)
    w1g = np.ascontiguousarray(
        ew1.astype(BF16).reshape(E, KD, P, KH, P).transpose(0, 2, 3, 1, 4))
    w2g = np.ascontiguousarray(
        ew2.astype(BF16).reshape(E, KH, P, ND, P).transpose(0, 2, 3, 1, 4))
    eb1g = np.ascontiguousarray(
        eb1.astype(np.float32).reshape(E * KH, P).T)
    in_maps = []
    for c in range(N_CORES):
        xp = np.zeros((CS, D), dtype=BF16)
        o = 0
        for e in range(E):
            tok = core_toks[c][e]
            xp[o:o + len(tok)] = xb[tok]
            o += qs[e]
        xg = np.ascontiguousarray(xp.T.reshape(KD, P, CS).transpose(1, 0, 2))
        in_maps.append({"xg": xg, "w1g": w1g, "w2g": w2g, "eb1g": eb1g})
    return in_maps


def host_routing(xf, gate_w, gate_b, dp_w1, dp_b1, dp_w2, dp_b2):
    """Exact fp64 routing: per-token k, top-k selection, masked softmax.

    fp64 matches the fp32 reference's discrete decisions unless a token sits
    within ~1e-6 of a boundary (none at these margins — nearest tokens are
    ~1e-4 away).
    """
    T = xf.shape[0]
    x64 = xf.astype(np.float64)
    h = np.maximum(x64 @ dp_w1.astype(np.float64) + dp_b1.astype(np.float64), 0)
    z = h @ dp_w2.astype(np.float64)[:, 0] + float(dp_b2[0])
    logits = x64 @ gate_w.astype(np.float64) + gate_b.astype(np.float64)[None, :]

    # k = round(1 + 3*clip((softplus(z)-0.5)/1.5, 0, 1)) via monotone
    # z-thresholds; middle threshold strict (RNE rounds kraw=2.5 down to 2)
    step = (TH_HI - TH_LO) / (2 * (E - MIN_E))
    zts = [np.log(np.expm1(TH_LO + (2 * i - 1) * step)) for i in (1, 2, 3)]
    k = 1 + (z >= zts[0]).astype(np.int64) + (z > zts[1]) + (z >= zts[2])

    order = np.argsort(-logits, axis=1, kind="stable")
    rank = np.empty_like(order)
    rank[np.arange(T)[:, None], order] = np.arange(E)[None, :]
    sel = rank < k[:, None]

    m = logits.max(axis=1, keepdims=True)
    ex = np.where(sel, np.exp(logits - m), 0.0)
    dense_w = ex / ex.sum(axis=1, keepdims=True)
    return dense_w


def dispatch(dense_w):
    """Group tokens by assigned expert, split each expert across 2 cores."""
    halves = []
    for e in range(E):
        idx = np.nonzero(dense_w[:, e] > 0)[0]
        h = (len(idx) + 1) // 2
        halves.append(idx[:h])
        halves.append(idx[h:])
    cmax = max((len(h) for h in halves), default=1)
    C = max(((cmax + 1) // 2) * 2, 512)
    return halves, C


def prep_ffn_in_maps(xf, halves, C, ew1, eb1, ew2):
    xb = xf.astype(BF16)
    w1ts = [np.ascontiguousarray(
        ew1[e].astype(BF16).reshape(KD, P, KH, P).transpose(1, 2, 0, 3))
        for e in range(E)]
    w2ts = [np.ascontiguousarray(
        ew2[e].astype(BF16).reshape(KH, P, ND, P).transpose(1, 2, 0, 3))
        for e in range(E)]
    eb1ts = [np.ascontiguousarray(eb1[e].astype(np.float32).reshape(KH, P).T)
             for e in range(E)]
    in_maps = []
    for c in range(N_CORES):
        e = c // 2
        tok = halves[c]
        xp = np.zeros((C, D), dtype=BF16)
        xp[:len(tok)] = xb[tok]
        xg = np.ascontiguousarray(xp.T.reshape(KD, P, C).transpose(1, 0, 2))
        in_maps.append({
            "xg": xg, "w1g": w1ts[e], "w2g": w2ts[e], "eb1g": eb1ts[e],
        })
    return in_maps


def remap_names(in_maps, names):
    return [{names[k]: v for k, v in m.items()} for m in in_maps]


_BUILT = {}


def _get(key, builder):
    if key not in _BUILT:
        _BUILT[key] = builder()
    return _BUILT[key]


def kernel(x, gate_w, gate_b, dp_w1, dp_b1, dp_w2, dp_b2, ew1, eb1, ew2, eb2,
           trace=False):
    from concourse.bass_utils import run_bass_kernel_spmd

    cores = list(range(N_CORES))
    xf = np.ascontiguousarray(
        np.asarray(x).reshape(T_FULL, D).astype(np.float32, copy=False))

    # ---- host: routing (fp64, exact) + balanced dispatch ----
    dense_w = host_routing(xf, gate_w, gate_b, dp_w1, dp_b1, dp_w2, dp_b2)
    core_toks, qs = dispatch_seg(dense_w)
    CS = sum(qs)

    # ---- device: expert FFNs (one SPMD launch, bf16) ----
    nc, names = _get(("ffnseg", qs), lambda: build_ffn_seg_nc(qs))
    in_maps = prep_ffn_seg_in_maps(xf, core_toks, qs, ew1, eb1, ew2)
    res = run_bass_kernel_spmd(nc, remap_names(in_maps, names), cores,
                               trace=trace)

    # ---- host: combine (scatter-add with routing weights) ----
    out = np.zeros((T_FULL, D), dtype=np.float64)
    for c in range(N_CORES):
        yt = res.results[c][names["yt"]]              # [P, ND, CS] bf16
        yg = yt.transpose(2, 1, 0).reshape(CS, D).astype(np.float64)
        o = 0
        for e in range(E):
            tok = core_toks[c][e]
            if len(tok):
                out[tok] += dense_w[tok, e, None] * yg[o:o + len(tok)]
            o += qs[e]
    if np.any(eb2):
        out += dense_w @ eb2.astype(np.float64)
    out = out.astype(np.float32).reshape(4, 2048, D)
    if trace:
        return out, (res,)
    return out


# revision 17
# speedup vs baseline: 1.1840x; 1.1840x over previous
"""Trainium2 Bass kernel for nn_ConditionalMoELayer (expert-parallel, sparse).

Design: the routing pipeline (difficulty net -> k, gate -> top-k softmax) is
tiny (~3% of FLOPs) and entirely sequential with the dispatch decision, so it
runs on the host in fp64 — bit-identical routing decisions to the fp32
reference (decision boundaries sit ~1e-1 apart while fp32-vs-fp64 z noise is
~1e-6). The host then groups tokens by assigned expert ("all-to-all"), and
ONE SPMD device launch runs the expert FFNs, which is >97% of the FLOPs:

  cores 2e, 2e+1 hold expert e's weights; each processes half of expert e's
  assigned tokens (C = max over experts of ceil(n_e/2), zero-padded).

The FFN runs in bf16 (matmul full rate, 1 cycle/row — same as fp32r at
>=256-wide moving operands, but half the HBM traffic and no small-tile rate
cliff on the tail chunk). W1 and W2 both stay resident in SBUF (64KB of the
208KB partition budget), so HBM reads are just 11.5MB/core vs compute of
~183us — DMA fully hidden. PSUM accumulates in fp32; only operand storage is
bf16, so the output error is ~3e-3 relative, far inside the 2e-2 gate.

Host combine: out[tok] += dense_w * y (token sets within one core are
disjoint), plus dense_w @ eb2.

Only ~40% of token-expert pairs are active, so this does ~2.5x fewer FLOPs
than the dense equivalent.
"""

import numpy as np
import ml_dtypes

BF16 = np.dtype(ml_dtypes.bfloat16)

P = 128          # partitions
D = 1024         # d_model
H = 2048         # expert hidden
E = 4            # experts
T_FULL = 8192    # total tokens
N_CORES = 8

KD = D // P      # 8   k-subtiles over D
KH = H // P      # 16  k-subtiles over H
ND = D // P      # 8   d-tiles (FFN output)
TCH = 512        # token chunk for matmul moving operand (1 PSUM bank of f32)

TH_LO = 0.5
TH_HI = 2.0
MIN_E = 1


def _ap_name(t):
    return t.tensor.name if hasattr(t, "tensor") else t.name


def build_ffn_nc(C):
    """One expert FFN over C gathered tokens per core, bf16 operands.

    mm1 computes hid^T = relu(W1^T x^T + b1), mm2 computes y^T = W2^T hid^T
    (weights stationary, tokens moving). W1, W2 fully resident in SBUF;
    x streams in per chunk on the gpsimd queue, y streams out behind it.
    """
    import concourse.mybir as mybir
    import concourse.tile as tile
    from concourse import bacc
    from contextlib import ExitStack

    f32 = mybir.dt.float32
    bf16 = mybir.dt.bfloat16
    AF = mybir.ActivationFunctionType

    # chunk widths: a 448-wide first chunk (small enough that its x lands
    # early and the first matmul fires ~12us in, large enough that mm1/mm2
    # consumption stays behind the weight stream), then 512s, with the
    # remainder arranged so no chunk is narrower than ~180 tokens (narrow
    # chunks approach the PE's LDWEIGHTS issue floor)
    widths = [min(448, C)]
    rem = C - widths[0]
    while rem > TCH + 180:
        widths.append(TCH)
        rem -= TCH
    if rem > TCH:
        widths.extend([(rem + 1) // 2, rem // 2])
    elif rem:
        widths.append(rem)
    chunks = []
    c0 = 0
    for cw in widths:
        chunks.append((c0, cw))
        c0 += cw

    nc = bacc.Bacc(None, target_bir_lowering=False, debug=False)

    with tile.TileContext(nc) as tc:
        with ExitStack() as ctx:
            dram = ctx.enter_context(tc.tile_pool(name="dram", bufs=1, space="DRAM"))
            xg_d = dram.tile([P, KD, C], bf16, kind="ExternalInput", name="xg")
            w1_d = dram.tile([P, KH, KD, P], bf16, kind="ExternalInput", name="w1g")
            w2_d = dram.tile([P, ND, KH, P], bf16, kind="ExternalInput", name="w2g")
            eb1_d = dram.tile([P, KH], f32, kind="ExternalInput", name="eb1g")
            yt_d = dram.tile([P, ND, C], bf16, kind="ExternalOutput", name="yt")

            const = ctx.enter_context(tc.tile_pool(name="const", bufs=1))
            xgp = ctx.enter_context(tc.tile_pool(name="xgp", bufs=1))
            hidp = ctx.enter_context(tc.tile_pool(name="hidp", bufs=2))
            ytp = ctx.enter_context(tc.tile_pool(name="ytp", bufs=2))
            psb = ctx.enter_context(tc.tile_pool(name="psb", bufs=3, space="PSUM"))
            ps2p = ctx.enter_context(tc.tile_pool(name="ps2p", bufs=3, space="PSUM"))

            w1_sb = const.tile([P, KH, KD, P], bf16, tag="w1")
            w2_sb = const.tile([P, ND, KH, P], bf16, tag="w2")
            eb1_sb = const.tile([P, KH], f32, tag="eb1")
            xg_sb = xgp.tile([P, KD, C], bf16, tag="xg")

            # DMA-completion semaphores post when the DATA actually lands,
            # and the two DMA queues carry independent bandwidth — so split
            # inputs across both, each in its consumers' order: activations
            # (and the tiny bias) on gpsimd, weights on sync. The first
            # matmul fires once xg chunk 0 + W1[ht0] land (~14us); after
            # that both streams stay ahead of the PE.
            (c00, cw0) = chunks[0]
            nc.sync.dma_start(xg_sb[:, :, :cw0], xg_d[:, :, :cw0])
            nc.gpsimd.dma_start(eb1_sb[:], eb1_d[:])
            for ht in range(KH):
                nc.sync.dma_start(w1_sb[:, ht], w1_d[:, ht])
            for dt in range(ND):
                nc.sync.dma_start(w2_sb[:, dt], w2_d[:, dt])
            for (c0, cw) in chunks[1:]:
                nc.gpsimd.dma_start(
                    xg_sb[:, :, c0:c0 + cw], xg_d[:, :, c0:c0 + cw]
                )

            for ci, (c0, cw) in enumerate(chunks):
                csl = slice(c0, c0 + cw)
                hidT = hidp.tile([P, KH, TCH], bf16, tag="hid")
                for ht in range(KH):
                    ps = psb.tile([P, TCH], f32, tag="psb")
                    for ko in range(KD):
                        nc.tensor.matmul(
                            ps[:, :cw],
                            w1_sb[:, ht, ko, :],
                            xg_sb[:, ko, csl],
                            start=(ko == 0),
                            stop=(ko == KD - 1),
                        )
                    nc.scalar.activation(
                        hidT[:, ht, :cw], ps[:, :cw], AF.Relu,
                        bias=eb1_sb[:, ht:ht + 1],
                    )
                last = ci == len(chunks) - 1
                yt_sb = ytp.tile([P, ND, TCH], bf16, tag="yt")
                for dt in range(ND):
                    ps2 = ps2p.tile([P, TCH], f32, tag="ps2")
                    for ko in range(KH):
                        nc.tensor.matmul(
                            ps2[:, :cw],
                            w2_sb[:, dt, ko, :],
                            hidT[:, ko, :cw],
                            start=(ko == 0),
                            stop=(ko == KH - 1),
                        )
                    nc.vector.tensor_copy(yt_sb[:, dt, :cw], ps2[:, :cw])
                    if last:
                        # tail: stream each dt-tile out as soon as it's cast
                        # so the final descriptor (and its completion post)
                        # is tiny; sync queue — its end-of-program drain is
                        # cheap where gpsimd's costs ~2.5us
                        nc.sync.dma_start(yt_d[:, dt, csl],
                                          yt_sb[:, dt, :cw])
                if not last:
                    nc.sync.dma_start(yt_d[:, :, csl], yt_sb[:, :, :cw])

    nc.compile()
    names = {k: _ap_name(v) for k, v in {
        "xg": xg_d, "w1g": w1_d, "w2g": w2_d, "eb1g": eb1_d, "yt": yt_d,
    }.items()}
    return nc, names


def build_ffn_seg_nc(qs):
    """Balanced variant: every core runs ALL experts, expert e over q_e =
    ceil(n_e/8) tokens (one <=512-wide chunk per expert segment). Expert
    weights stream per segment through double-buffered SBUF tiles; the
    sync-queue descriptor for segment e+2's W1 parks until segment e's mm1
    frees its buffer, which is harmless because it still lands a full
    segment early. Per-core tokens drop from max_e ceil(n_e/2) to
    sum_e ceil(n_e/8) — perfect load balance across the 8 cores.
    """
    import concourse.mybir as mybir
    import concourse.tile as tile
    from concourse import bacc
    from contextlib import ExitStack

    f32 = mybir.dt.float32
    bf16 = mybir.dt.bfloat16
    AF = mybir.ActivationFunctionType

    CS = sum(qs)
    offs = [sum(qs[:e]) for e in range(E)]

    nc = bacc.Bacc(None, target_bir_lowering=False, debug=False)

    with tile.TileContext(nc) as tc:
        with ExitStack() as ctx:
            dram = ctx.enter_context(tc.tile_pool(name="dram", bufs=1, space="DRAM"))
            xg_d = dram.tile([P, KD, CS], bf16, kind="ExternalInput", name="xg")
            w1_d = dram.tile([E, P, KH, KD, P], bf16, kind="ExternalInput", name="w1g")
            w2_d = dram.tile([E, P, ND, KH, P], bf16, kind="ExternalInput", name="w2g")
            eb1_d = dram.tile([P, E * KH], f32, kind="ExternalInput", name="eb1g")
            yt_d = dram.tile([P, ND, CS], bf16, kind="ExternalOutput", name="yt")

            const = ctx.enter_context(tc.tile_pool(name="const", bufs=1))
            w1p = ctx.enter_context(tc.tile_pool(name="w1p", bufs=2))
            w2p = ctx.enter_context(tc.tile_pool(name="w2p", bufs=2))
            xgp = ctx.enter_context(tc.tile_pool(name="xgp", bufs=1))
            hidp = ctx.enter_context(tc.tile_pool(name="hidp", bufs=1))
            ytp = ctx.enter_context(tc.tile_pool(name="ytp", bufs=2))
            psb = ctx.enter_context(tc.tile_pool(name="psb", bufs=3, space="PSUM"))
            ps2p = ctx.enter_context(tc.tile_pool(name="ps2p", bufs=3, space="PSUM"))
            wps = ctx.enter_context(tc.tile_pool(name="wps", bufs=1, space="PSUM"))

            eb1_sb = const.tile([P, E * KH], f32, tag="eb1")
            xg_sb = xgp.tile([P, KD, CS], bf16, tag="xg")
            w1ts = [w1p.tile([P, KH, KD, P], bf16, tag="w1s", name=f"w1s{e}")
                    for e in range(E)]
            w2ts = [w2p.tile([P, ND, KH, P], bf16, tag="w2s", name=f"w2s{e}")
                    for e in range(E)]

            # sync queue: segment-0 x first (gates the first matmul), then
            # weights in consumption order; y-outs are appended behind by
            # the compute loop. gpsimd queue: bias + remaining x segments.
            nc.sync.dma_start(xg_sb[:, :, :qs[0]], xg_d[:, :, :qs[0]])
            nc.gpsimd.dma_start(eb1_sb[:], eb1_d[:])
            for e in range(E):
                for ht in range(KH):
                    nc.sync.dma_start(w1ts[e][:, ht], w1_d[e, :, ht])
                for dt in range(ND):
                    nc.sync.dma_start(w2ts[e][:, dt], w2_d[e, :, dt])
            for e in range(1, E):
                sl = slice(offs[e], offs[e] + qs[e])
                nc.gpsimd.dma_start(xg_sb[:, :, sl], xg_d[:, :, sl])

            # Warm the PE's p-state during the ~7us wait for the first x/W1
            # data: the clock ramps to 2.4GHz only after ~3us of continuous
            # busy, so burn that ramp on throwaway matmuls over a memset
            # tile. Real matmuls then start at full rate.
            warm_sb = const.tile([P, 16], bf16, tag="warm")
            nc.gpsimd.memset(warm_sb[:], 0.0)
            psw = wps.tile([16, 16], f32, tag="psw")
            NWARM = 48
            for i in range(NWARM):
                # one long accumulation group — no completion event between
                # matmuls, so they issue back-to-back
                nc.tensor.matmul(psw, warm_sb[:], warm_sb[:],
                                 start=(i == 0), stop=(i == NWARM - 1))

            for e in range(E):
                c0, cw = offs[e], qs[e]
                csl = slice(c0, c0 + cw)
                last = e == E - 1
                hidT = hidp.tile([P, KH, TCH], bf16, tag="hid")
                for ht in range(KH):
                    ps = psb.tile([P, TCH], f32, tag="psb")
                    for ko in range(KD):
                        nc.tensor.matmul(
                            ps[:, :cw],
                            w1ts[e][:, ht, ko, :],
                            xg_sb[:, ko, csl],
                            start=(ko == 0),
                            stop=(ko == KD - 1),
                        )
                    nc.scalar.activation(
                        hidT[:, ht, :cw], ps[:, :cw], AF.Relu,
                        bias=eb1_sb[:, e * KH + ht:e * KH + ht + 1],
                    )
                yt_sb = ytp.tile([P, ND, TCH], bf16, tag="yt")
                for dt in range(ND):
                    ps2 = ps2p.tile([P, TCH], f32, tag="ps2")
                    for ko in range(KH):
                        nc.tensor.matmul(
                            ps2[:, :cw],
                            w2ts[e][:, dt, ko, :],
                            hidT[:, ko, :cw],
                            start=(ko == 0),
                            stop=(ko == KH - 1),
                        )
                    nc.vector.tensor_copy(yt_sb[:, dt, :cw], ps2[:, :cw])
                    if last:
                        nc.sync.dma_start(yt_d[:, dt, csl],
                                          yt_sb[:, dt, :cw])
                if not last:
                    nc.sync.dma_start(yt_d[:, :, csl], yt_sb[:, :, :cw])

    nc.compile()
    names = {k: _ap_name(v) for k, v in {
        "xg": xg_d, "w1g": w1_d, "w2g": w2_d, "eb1g": eb1_d, "yt": yt_d,
    }.items()}
    return nc, names


def dispatch_seg(dense_w):
    """Split every expert's token list evenly across all 8 cores."""
    qs = []
    slices = []
    for e in range(E):
        idx = np.nonzero(dense_w[:, e] > 0)[0]
        q = max((len(idx) + N_CORES - 1) // N_CORES, 16)
        qs.append(q)
        slices.append([idx[c * q:(c + 1) * q] for c in range(N_CORES)])
    core_toks = [[slices[e][c] for e in range(E)] for c in range(N_CORES)]
    return core_toks, tuple(qs)


def prep_ffn_seg_in_maps(xf, core_toks, qs, ew1, eb1, ew2):
    xb = xf.astype(BF16)
    CS = sum(qs)
    w1g = np.ascontiguousarray(
        ew1.astype(BF16).reshape(E, KD, P, KH, P).transpose(0, 2, 3, 1, 4))
    w2g = np.ascontiguousarray(
        ew2.astype(BF16).reshape(E, KH, P, ND, P).transpose(0, 2, 3, 1, 4))
    eb1g = np.ascontiguousarray(
        eb1.astype(np.float32).reshape(E * KH, P).T)
    in_maps = []
    for c in range(N_CORES):
        xp = np.zeros((CS, D), dtype=BF16)
        o = 0
        for e in range(E):
            tok = core_toks[c][e]
            xp[o:o + len(tok)] = xb[tok]
            o += qs[e]
        xg = np.ascontiguousarray(xp.T.reshape(KD, P, CS).transpose(1, 0, 2))
        in_maps.append({"xg": xg, "w1g": w1g, "w2g": w2g, "eb1g": eb1g})
    return in_maps


def host_routing(xf, gate_w, gate_b, dp_w1, dp_b1, dp_w2, dp_b2):
    """Exact fp64 routing: per-token k, top-k selection, masked softmax.

    fp64 matches the fp32 reference's discrete decisions unless a token sits
    within ~1e-6 of a boundary (none at these margins — nearest tokens are
    ~1e-4 away).
    """
    T = xf.shape[0]
    x64 = xf.astype(np.float64)
    h = np.maximum(x64 @ dp_w1.astype(np.float64) + dp_b1.astype(np.float64), 0)
    z = h @ dp_w2.astype(np.float64)[:, 0] + float(dp_b2[0])
    logits = x64 @ gate_w.astype(np.float64) + gate_b.astype(np.float64)[None, :]

    # k = round(1 + 3*clip((softplus(z)-0.5)/1.5, 0, 1)) via monotone
    # z-thresholds; middle threshold strict (RNE rounds kraw=2.5 down to 2)
    step = (TH_HI - TH_LO) / (2 * (E - MIN_E))
    zts = [np.log(np.expm1(TH_LO + (2 * i - 1) * step)) for i in (1, 2, 3)]
    k = 1 + (z >= zts[0]).astype(np.int64) + (z > zts[1]) + (z >= zts[2])

    order = np.argsort(-logits, axis=1, kind="stable")
    rank = np.empty_like(order)
    rank[np.arange(T)[:, None], order] = np.arange(E)[None, :]
    sel = rank < k[:, None]

    m = logits.max(axis=1, keepdims=True)
    ex = np.where(sel, np.exp(logits - m), 0.0)
    dense_w = ex / ex.sum(axis=1, keepdims=True)
    return dense_w


def dispatch(dense_w):
    """Group tokens by assigned expert, split each expert across 2 cores."""
    halves = []
    for e in range(E):
        idx = np.nonzero(dense_w[:, e] > 0)[0]
        h = (len(idx) + 1) // 2
        halves.append(idx[:h])
        halves.append(idx[h:])
    cmax = max((len(h) for h in halves), default=1)
    C = max(((cmax + 1) // 2) * 2, 512)
    return halves, C


def prep_ffn_in_maps(xf, halves, C, ew1, eb1, ew2):
    xb = xf.astype(BF16)
    w1ts = [np.ascontiguousarray(
        ew1[e].astype(BF16).reshape(KD, P, KH, P).transpose(1, 2, 0, 3))
        for e in range(E)]
    w2ts = [np.ascontiguousarray(
        ew2[e].astype(BF16).reshape(KH, P, ND, P).transpose(1, 2, 0, 3))
        for e in range(E)]
    eb1ts = [np.ascontiguousarray(eb1[e].astype(np.float32).reshape(KH, P).T)
             for e in range(E)]
    in_maps = []
    for c in range(N_CORES):
        e = c // 2
        tok = halves[c]
        xp = np.zeros((C, D), dtype=BF16)
        xp[:len(tok)] = xb[tok]
        xg = np.ascontiguousarray(xp.T.reshape(KD, P, C).transpose(1, 0, 2))
        in_maps.append({
            "xg": xg, "w1g": w1ts[e], "w2g": w2ts[e], "eb1g": eb1ts[e],
        })
    return in_maps


def remap_names(in_maps, names):
    return [{names[k]: v for k, v in m.items()} for m in in_maps]


_BUILT = {}


def _get(key, builder):
    if key not in _BUILT:
        _BUILT[key] = builder()
    return _BUILT[key]


def kernel(x, gate_w, gate_b, dp_w1, dp_b1, dp_w2, dp_b2, ew1, eb1, ew2, eb2,
           trace=False):
    from concourse.bass_utils import run_bass_kernel_spmd

    cores = list(range(N_CORES))
    xf = np.ascontiguousarray(
        np.asarray(x).reshape(T_FULL, D).astype(np.float32, copy=False))

    # ---- host: routing (fp64, exact) + balanced dispatch ----
    dense_w = host_routing(xf, gate_w, gate_b, dp_w1, dp_b1, dp_w2, dp_b2)
    core_toks, qs = dispatch_seg(dense_w)
    CS = sum(qs)

    # ---- device: expert FFNs (one SPMD launch, bf16) ----
    nc, names = _get(("ffnseg", qs), lambda: build_ffn_seg_nc(qs))
    in_maps = prep_ffn_seg_in_maps(xf, core_toks, qs, ew1, eb1, ew2)
    res = run_bass_kernel_spmd(nc, remap_names(in_maps, names), cores,
                               trace=trace)

    # ---- host: combine (scatter-add with routing weights) ----
    out = np.zeros((T_FULL, D), dtype=np.float64)
    for c in range(N_CORES):
        yt = res.results[c][names["yt"]]              # [P, ND, CS] bf16
        yg = yt.transpose(2, 1, 0).reshape(CS, D).astype(np.float64)
        o = 0
        for e in range(E):
            tok = core_toks[c][e]
            if len(tok):
                out[tok] += dense_w[tok, e, None] * yg[o:o + len(tok)]
            o += qs[e]
    if np.any(eb2):
        out += dense_w @ eb2.astype(np.float64)
    out = out.astype(np.float32).reshape(4, 2048, D)
    if trace:
        return out, (res,)
    return out
